# revision 36
# baseline (speedup 1.0000x reference)
"""CKA loss kernel for Trainium2 (8 NeuronCores, SPMD).

Math: for X_i = output[i] (shape [N, D]), the reference builds N x N Gram
matrices K_i = X_i X_i^T (diag zeroed), double-centers them and takes
pairwise inner products. Algebraically this reduces to D x D cross-Grams:

    S[i,j] = <Khat_i, Khat_j>
           = ||X_i^T X_j||_F^2 - g_i.g_j - (2/N) r_i.r_j + t_i t_j / N^2

with g_i[n] = ||X_i[n]||^2, r_i = X_i (X_i^T 1) - g_i, t_i = sum(r_i).
Only the ||X_i^T X_j||_F^2 term is heavy (C_ij = X_i^T X_j, 512x512,
contraction over N=4096) — everything else is O(M N D) and done on host
in float64.

Sharding: the contraction axis N is split across the 8 cores (512 samples
each). Every core computes partial C_ij for all 21 pairs (i<=j) from its
slice — perfectly balanced, no collectives. The host sums the partials in
float64, squares, and assembles the tiny [6,6] output.

The matmuls run in float16 (inputs are ~N(0,1); fp16 rounding of the
inputs perturbs S by ~1e-5 relative — the final cancellation is only ~8x).
Partial C blocks return to the host as float16 as well (entries are
O(100), and the per-entry rounding is ~2^-11 relative, far below the
tolerance). For diagonal pairs C_ii only column-blocks e >= d-block are
computed (block-level symmetry), saving ~12% of the matmul work.
"""

import numpy as np

M, N, D = 6, 4096, 512
NCORES = 8
NLOC = N // NCORES   # samples per core
P = 128              # partitions
NCH = NLOC // P      # contraction chunks per core
KBLK = D // P        # 128-row blocks of C
PAIRS = [(i, j) for i in range(M) for j in range(i, M)]  # 21
EPS = 1e-6

MM_DTYPE = "float16"  # matmul operand/output-staging dtype

_CACHE = {}


def _np_mm_dtype():
    if MM_DTYPE == "float16":
        return np.float16
    import ml_dtypes

    if MM_DTYPE == "bfloat16":
        return np.dtype(ml_dtypes.bfloat16)
    if MM_DTYPE == "float8e4":
        return np.dtype(ml_dtypes.float8_e4m3)
    raise ValueError(MM_DTYPE)


def _build_nc(mm_dtype_name=None):
    mm_dtype_name = mm_dtype_name or MM_DTYPE
    import concourse.bacc as bacc
    import concourse.mybir as mybir
    import concourse.tile as tile

    mm_dt = getattr(mybir.dt, mm_dtype_name)
    fp8 = mm_dtype_name == "float8e4"
    # fp8 runs DoubleRow: 2 contraction sub-rows per partition per matmul
    ks = 2 if fp8 else 1
    nch = NLOC // (P * ks)  # contraction chunks per core
    nc = bacc.Bacc("TRN2", target_bir_lowering=False, debug=False,
                   num_devices=NCORES)
    # chunk-major input: each (layer, chunk) slice is one contiguous
    # region -> large DMA descriptors. Layout [P, ks, D] per chunk.
    x_in = nc.dram_tensor("x", [M, nch, P, ks, D], mm_dt,
                          kind="ExternalInput")
    cout = nc.dram_tensor("cout", [len(PAIRS), P, KBLK * D],
                          mybir.dt.float16, kind="ExternalOutput")

    perf_mode = mybir.MatmulPerfMode.DoubleRow if fp8 else None

    with tile.TileContext(nc) as tc:
        with (
            tc.tile_pool(name="xin", bufs=1) as xpool,
            tc.tile_pool(name="stage", bufs=3) as spool,
            tc.tile_pool(name="psum", bufs=8, space="PSUM") as ppool,
        ):
            # warmup: the PE sits idle ~3.5us waiting for the first input
            # DMA, and the HAM clock-gate needs >3us of sustained activity
            # to open to full clock. Run dummy matmuls on a zeroed scratch
            # tile during that window so the real stream starts warm.
            wsrc = xpool.tile([P, D], mm_dt, tag="warm")
            nc.gpsimd.memset(wsrc[:], 0.0)
            wacc = ppool.tile([P, D], mybir.dt.float32, tag="acc")
            for _ in range(8):
                nc.tensor.matmul(wacc[:], wsrc[:, :P], wsrc[:],
                                 start=True, stop=True)

            # one tile per (layer, chunk) so matmuls wait only on the
            # chunks they actually read — faster pipeline ramp
            xs = []
            ndma = 0
            # rotate triggers over the three DMA-capable engines: each
            # DMA_DIRECT2D costs ~0.65us of issuing-engine queue time,
            # and the early stream is input-arrival limited
            trig = [nc.sync, nc.gpsimd, nc.scalar]
            for i in range(M):
                row = []
                for c in range(nch):
                    xt = xpool.tile([P, ks, D], mm_dt, tag=f"x{i}c{c}")
                    trig[ndma % 3].dma_start(xt[:], x_in[i, c])
                    ndma += 1
                    row.append(xt)
                xs.append(row)

            ncopy = 0
            for pi, (i, j) in enumerate(PAIRS):
                last_pair = pi == len(PAIRS) - 1
                st = spool.tile([P, KBLK * D], mybir.dt.float16, tag="st")
                off = 0
                for k in range(KBLK):
                    # C_ij rows [128k, 128k+128); for i==j skip column
                    # blocks left of the diagonal block.
                    e0 = k * P if i == j else 0
                    w = D - e0
                    acc = ppool.tile([P, D], mybir.dt.float32, tag="acc")
                    for c in range(nch):
                        nc.tensor.matmul(
                            acc[:, :w],
                            xs[i][c][:, :, k * P:(k + 1) * P],
                            xs[j][c][:, :, e0:],
                            start=(c == 0),
                            stop=(c == nch - 1),
                            perf_mode=perf_mode,
                        )
                    # spread PSUM->SBUF cast-copies over DVE and ACT
                    # (keep the final pair on DVE: lower latency tail)
                    dst = st[:, off:off + w]
                    if ncopy % 3 == 2 and not last_pair:
                        nc.scalar.copy(dst, acc[:, :w])
                    else:
                        nc.vector.tensor_copy(dst, acc[:, :w])
                    ncopy += 1
                    if last_pair:
                        # final pair: DMA each block as its copy lands,
                        # from the (idle-by-now) GpSimd queue, so most of
                        # the output drains during the remaining matmuls
                        nc.gpsimd.dma_start(cout[pi, :, off:off + w], dst)
                    off += w
                if not last_pair:
                    nc.sync.dma_start(cout[pi, :, :off], st[:, :off])

    nc.compile()
    return nc


def _get_nc():
    if MM_DTYPE not in _CACHE:
        _CACHE[MM_DTYPE] = _build_nc()
    return _CACHE[MM_DTYPE]


def _shard_inputs(X):
    """X: [M, N, D] float32 -> per-core low-precision [M, nch, P, ks, D].

    Contraction sample (within a chunk) maps to (ks*128 + p) for fp8
    DoubleRow (ks=2), or just p otherwise (ks=1)."""
    Xh = X.astype(_np_mm_dtype())
    ks = 2 if MM_DTYPE == "float8e4" else 1
    nch = NLOC // (P * ks)
    maps = []
    for c in range(NCORES):
        sl = Xh[:, c * NLOC:(c + 1) * NLOC, :]
        sl = sl.reshape(M, nch, ks, P, D).transpose(0, 1, 3, 2, 4)
        maps.append({"x": np.ascontiguousarray(sl)})
    return maps


def _ensure_axon_hooks():
    """Some images lack antenv.axon_hooks; bass_utils imports it whenever
    tracing is requested (including via the BASS_TRACE env var). Provide a
    null hook module so that path degrades gracefully instead of raising."""
    try:
        from antenv.axon_hooks import get_axon_ntff_profile_hook  # noqa: F401
        return
    except ImportError:
        pass
    import sys
    import types

    try:
        import antenv
    except ImportError:
        return
    mod = types.ModuleType("antenv.axon_hooks")
    mod._hook = None
    mod.set_axon_ntff_profile_hook = lambda h: setattr(mod, "_hook", h)
    mod.get_axon_ntff_profile_hook = lambda: mod._hook
    sys.modules["antenv.axon_hooks"] = mod
    antenv.axon_hooks = mod


def _run_device(X, trace=False):
    _ensure_axon_hooks()
    from concourse.bass_utils import run_bass_kernel_spmd

    nc = _get_nc()
    res = run_bass_kernel_spmd(nc, _shard_inputs(X),
                               core_ids=list(range(NCORES)), trace=trace)
    return res


def _assemble(X, couts):
    """Host-side float64 assembly of the final CKA outputs.

    couts: per-core [21, P, KBLK*D] arrays of partial C blocks; block k
    of pair (i,j) has width w_k (= D, or D - 128k for diagonal pairs)
    and the blocks are packed back-to-back from column 0.
    """
    Csum = np.zeros((len(PAIRS), P, KBLK * D), np.float64)
    for c in couts:
        Csum += c.astype(np.float64)

    F2 = np.zeros((M, M))
    for pi, (i, j) in enumerate(PAIRS):
        if i != j:
            v = np.sum(Csum[pi] ** 2)
        else:
            v = 0.0
            off = 0
            for k in range(KBLK):
                w = D - k * P
                row = Csum[pi, :, off:off + w]
                v += np.sum(row[:, :P] ** 2)            # diagonal block (k,k)
                if w > P:
                    v += 2.0 * np.sum(row[:, P:] ** 2)  # blocks l > k
                off += w
        F2[i, j] = v
        F2[j, i] = v

    # use the same rounded X the device matmuls consumed, so the
    # g/r/t terms cancel the diagonal contributions of ||C||^2 exactly
    Xd = X.astype(_np_mm_dtype()).astype(np.float64)
    g = np.einsum("ind,ind->in", Xd, Xd)            # [M, N]
    s = Xd.sum(axis=1)                              # [M, D]
    r = np.einsum("ind,id->in", Xd, s) - g          # [M, N]
    t = r.sum(axis=1)                               # [M]

    S = F2 - g @ g.T - (2.0 / N) * (r @ r.T) + np.outer(t, t) / N ** 2
    S = np.abs(S)
    diag = np.diagonal(S)
    cka = S / np.sqrt(diag[:, None] * diag[None, :] + EPS)
    hsic = cka * np.tril(np.ones((M, M)), k=-1)
    l = np.sum(np.abs(hsic))
    vis = hsic + hsic.T + np.eye(M)
    return vis.astype(np.float32), np.array(l, dtype=np.float32)


def kernel(output):
    X = np.ascontiguousarray(np.asarray(output), dtype=np.float32)
    assert X.shape == (M, N, D)
    res = _run_device(X)
    couts = [r["cout"] for r in res.results]
    return _assemble(X, couts)


# revision 37
# speedup vs baseline: 1.0071x; 1.0071x over previous
"""CKA loss kernel for Trainium2 (8 NeuronCores, SPMD).

Math: for X_i = output[i] (shape [N, D]), the reference builds N x N Gram
matrices K_i = X_i X_i^T (diag zeroed), double-centers them and takes
pairwise inner products. Algebraically this reduces to D x D cross-Grams:

    S[i,j] = <Khat_i, Khat_j>
           = ||X_i^T X_j||_F^2 - g_i.g_j - (2/N) r_i.r_j + t_i t_j / N^2

with g_i[n] = ||X_i[n]||^2, r_i = X_i (X_i^T 1) - g_i, t_i = sum(r_i).
Only the ||X_i^T X_j||_F^2 term is heavy (C_ij = X_i^T X_j, 512x512,
contraction over N=4096) — everything else is O(M N D) and done on host
in float64.

Sharding: the contraction axis N is split across the 8 cores (512 samples
each). Every core computes partial C_ij for all 21 pairs (i<=j) from its
slice — perfectly balanced, no collectives. The host sums the partials in
float64, squares, and assembles the tiny [6,6] output.

The matmuls run in float16 (inputs are ~N(0,1); fp16 rounding of the
inputs perturbs S by ~1e-5 relative — the final cancellation is only ~8x).
Partial C blocks return to the host as float16 as well (entries are
O(100), and the per-entry rounding is ~2^-11 relative, far below the
tolerance). For diagonal pairs C_ii only column-blocks e >= d-block are
computed (block-level symmetry), saving ~12% of the matmul work.
"""

import numpy as np

M, N, D = 6, 4096, 512
NCORES = 8
NLOC = N // NCORES   # samples per core
P = 128              # partitions
NCH = NLOC // P      # contraction chunks per core
KBLK = D // P        # 128-row blocks of C
PAIRS = [(i, j) for i in range(M) for j in range(i, M)]  # 21
EPS = 1e-6

MM_DTYPE = "float16"  # matmul operand/output-staging dtype

_CACHE = {}


def _np_mm_dtype():
    if MM_DTYPE == "float16":
        return np.float16
    import ml_dtypes

    if MM_DTYPE == "bfloat16":
        return np.dtype(ml_dtypes.bfloat16)
    if MM_DTYPE == "float8e4":
        return np.dtype(ml_dtypes.float8_e4m3)
    raise ValueError(MM_DTYPE)


def _build_nc(mm_dtype_name=None):
    mm_dtype_name = mm_dtype_name or MM_DTYPE
    import concourse.bacc as bacc
    import concourse.mybir as mybir
    import concourse.tile as tile

    mm_dt = getattr(mybir.dt, mm_dtype_name)
    fp8 = mm_dtype_name == "float8e4"
    # fp8 runs DoubleRow: 2 contraction sub-rows per partition per matmul
    ks = 2 if fp8 else 1
    nch = NLOC // (P * ks)  # contraction chunks per core
    nc = bacc.Bacc("TRN2", target_bir_lowering=False, debug=False,
                   num_devices=NCORES)
    # chunk-major input: each (layer, chunk) slice is one contiguous
    # region -> large DMA descriptors. Layout [P, ks, D] per chunk.
    x_in = nc.dram_tensor("x", [M, nch, P, ks, D], mm_dt,
                          kind="ExternalInput")
    cout = nc.dram_tensor("cout", [len(PAIRS), P, KBLK * D],
                          mybir.dt.float16, kind="ExternalOutput")

    perf_mode = mybir.MatmulPerfMode.DoubleRow if fp8 else None

    with tile.TileContext(nc) as tc:
        with (
            tc.tile_pool(name="xin", bufs=1) as xpool,
            tc.tile_pool(name="stage", bufs=3) as spool,
            tc.tile_pool(name="psum", bufs=8, space="PSUM") as ppool,
        ):
            # warmup: the PE sits idle ~3.5us waiting for the first input
            # DMA, and the HAM clock-gate needs >3us of sustained activity
            # to open to full clock. Run dummy matmuls on a zeroed scratch
            # tile during that window so the real stream starts warm.
            wsrc = xpool.tile([P, D], mm_dt, tag="warm")
            nc.gpsimd.memset(wsrc[:], 0.0)
            wacc = ppool.tile([P, D], mybir.dt.float32, tag="acc")
            for _ in range(8):
                nc.tensor.matmul(wacc[:], wsrc[:, :P], wsrc[:],
                                 start=True, stop=True)

            # one tile per (layer, chunk) so matmuls wait only on the
            # chunks they actually read — faster pipeline ramp
            xs = []
            ndma = 0
            # rotate triggers over the three DMA-capable engines: each
            # DMA_DIRECT2D costs ~0.65us of issuing-engine queue time,
            # and the early stream is input-arrival limited
            trig = [nc.sync, nc.gpsimd, nc.scalar]
            for i in range(M):
                row = []
                for c in range(nch):
                    xt = xpool.tile([P, ks, D], mm_dt, tag=f"x{i}c{c}")
                    trig[ndma % 3].dma_start(xt[:], x_in[i, c])
                    ndma += 1
                    row.append(xt)
                xs.append(row)

            ncopy = 0
            for pi, (i, j) in enumerate(PAIRS):
                st = spool.tile([P, KBLK * D], mybir.dt.float16, tag="st")
                off = 0
                for k in range(KBLK):
                    # C_ij rows [128k, 128k+128); for i==j skip column
                    # blocks left of the diagonal block.
                    e0 = k * P if i == j else 0
                    w = D - e0
                    acc = ppool.tile([P, D], mybir.dt.float32, tag="acc")
                    for c in range(nch):
                        nc.tensor.matmul(
                            acc[:, :w],
                            xs[i][c][:, :, k * P:(k + 1) * P],
                            xs[j][c][:, :, e0:],
                            start=(c == 0),
                            stop=(c == nch - 1),
                            perf_mode=perf_mode,
                        )
                    # spread PSUM->SBUF cast-copies over DVE and ACT
                    # (keep the final pair on DVE: lower latency tail)
                    dst = st[:, off:off + w]
                    last_pair = pi == len(PAIRS) - 1
                    if ncopy % 3 == 2 and not last_pair:
                        nc.scalar.copy(dst, acc[:, :w])
                    else:
                        nc.vector.tensor_copy(dst, acc[:, :w])
                    ncopy += 1
                    off += w
                nc.sync.dma_start(cout[pi, :, :off], st[:, :off])

    nc.compile()
    return nc


def _get_nc():
    if MM_DTYPE not in _CACHE:
        _CACHE[MM_DTYPE] = _build_nc()
    return _CACHE[MM_DTYPE]


def _shard_inputs(X):
    """X: [M, N, D] float32 -> per-core low-precision [M, nch, P, ks, D].

    Contraction sample (within a chunk) maps to (ks*128 + p) for fp8
    DoubleRow (ks=2), or just p otherwise (ks=1)."""
    Xh = X.astype(_np_mm_dtype())
    ks = 2 if MM_DTYPE == "float8e4" else 1
    nch = NLOC // (P * ks)
    maps = []
    for c in range(NCORES):
        sl = Xh[:, c * NLOC:(c + 1) * NLOC, :]
        sl = sl.reshape(M, nch, ks, P, D).transpose(0, 1, 3, 2, 4)
        maps.append({"x": np.ascontiguousarray(sl)})
    return maps


def _ensure_axon_hooks():
    """Some images lack antenv.axon_hooks; bass_utils imports it whenever
    tracing is requested (including via the BASS_TRACE env var). Provide a
    null hook module so that path degrades gracefully instead of raising."""
    try:
        from antenv.axon_hooks import get_axon_ntff_profile_hook  # noqa: F401
        return
    except ImportError:
        pass
    import sys
    import types

    try:
        import antenv
    except ImportError:
        return
    mod = types.ModuleType("antenv.axon_hooks")
    mod._hook = None
    mod.set_axon_ntff_profile_hook = lambda h: setattr(mod, "_hook", h)
    mod.get_axon_ntff_profile_hook = lambda: mod._hook
    sys.modules["antenv.axon_hooks"] = mod
    antenv.axon_hooks = mod


def _run_device(X, trace=False):
    _ensure_axon_hooks()
    from concourse.bass_utils import run_bass_kernel_spmd

    nc = _get_nc()
    res = run_bass_kernel_spmd(nc, _shard_inputs(X),
                               core_ids=list(range(NCORES)), trace=trace)
    return res


def _assemble(X, couts):
    """Host-side float64 assembly of the final CKA outputs.

    couts: per-core [21, P, KBLK*D] arrays of partial C blocks; block k
    of pair (i,j) has width w_k (= D, or D - 128k for diagonal pairs)
    and the blocks are packed back-to-back from column 0.
    """
    Csum = np.zeros((len(PAIRS), P, KBLK * D), np.float64)
    for c in couts:
        Csum += c.astype(np.float64)

    F2 = np.zeros((M, M))
    for pi, (i, j) in enumerate(PAIRS):
        if i != j:
            v = np.sum(Csum[pi] ** 2)
        else:
            v = 0.0
            off = 0
            for k in range(KBLK):
                w = D - k * P
                row = Csum[pi, :, off:off + w]
                v += np.sum(row[:, :P] ** 2)            # diagonal block (k,k)
                if w > P:
                    v += 2.0 * np.sum(row[:, P:] ** 2)  # blocks l > k
                off += w
        F2[i, j] = v
        F2[j, i] = v

    # use the same rounded X the device matmuls consumed, so the
    # g/r/t terms cancel the diagonal contributions of ||C||^2 exactly
    Xd = X.astype(_np_mm_dtype()).astype(np.float64)
    g = np.einsum("ind,ind->in", Xd, Xd)            # [M, N]
    s = Xd.sum(axis=1)                              # [M, D]
    r = np.einsum("ind,id->in", Xd, s) - g          # [M, N]
    t = r.sum(axis=1)                               # [M]

    S = F2 - g @ g.T - (2.0 / N) * (r @ r.T) + np.outer(t, t) / N ** 2
    S = np.abs(S)
    diag = np.diagonal(S)
    cka = S / np.sqrt(diag[:, None] * diag[None, :] + EPS)
    hsic = cka * np.tril(np.ones((M, M)), k=-1)
    l = np.sum(np.abs(hsic))
    vis = hsic + hsic.T + np.eye(M)
    return vis.astype(np.float32), np.array(l, dtype=np.float32)


def kernel(output):
    X = np.ascontiguousarray(np.asarray(output), dtype=np.float32)
    assert X.shape == (M, N, D)
    res = _run_device(X)
    couts = [r["cout"] for r in res.results]
    return _assemble(X, couts)


# revision 39
# speedup vs baseline: 1.0239x; 1.0166x over previous
"""CKA loss kernel for Trainium2 (8 NeuronCores, SPMD).

Math: for X_i = output[i] (shape [N, D]), the reference builds N x N Gram
matrices K_i = X_i X_i^T (diag zeroed), double-centers them and takes
pairwise inner products. Algebraically this reduces to D x D cross-Grams:

    S[i,j] = <Khat_i, Khat_j>
           = ||X_i^T X_j||_F^2 - g_i.g_j - (2/N) r_i.r_j + t_i t_j / N^2

with g_i[n] = ||X_i[n]||^2, r_i = X_i (X_i^T 1) - g_i, t_i = sum(r_i).
Only the ||X_i^T X_j||_F^2 term is heavy (C_ij = X_i^T X_j, 512x512,
contraction over N=4096) — everything else is O(M N D) and done on host
in float64.

Sharding: the contraction axis N is split across the 8 cores (512 samples
each). Every core computes partial C_ij for all 21 pairs (i<=j) from its
slice — perfectly balanced, no collectives. The host sums the partials in
float64, squares, and assembles the tiny [6,6] output.

The matmuls run in float16 (inputs are ~N(0,1); fp16 rounding of the
inputs perturbs S by ~1e-5 relative — the final cancellation is only ~8x).
Partial C blocks return to the host as float16 as well (entries are
O(100), and the per-entry rounding is ~2^-11 relative, far below the
tolerance). For diagonal pairs C_ii only column-blocks e >= d-block are
computed (block-level symmetry), saving ~12% of the matmul work.
"""

import numpy as np

M, N, D = 6, 4096, 512
NCORES = 8
NLOC = N // NCORES   # samples per core
P = 128              # partitions
NCH = NLOC // P      # contraction chunks per core
KBLK = D // P        # 128-row blocks of C
PAIRS = [(i, j) for i in range(M) for j in range(i, M)]  # 21
EPS = 1e-6

MM_DTYPE = "float16"  # matmul operand/output-staging dtype

_CACHE = {}


def _np_mm_dtype():
    if MM_DTYPE == "float16":
        return np.float16
    import ml_dtypes

    if MM_DTYPE == "bfloat16":
        return np.dtype(ml_dtypes.bfloat16)
    if MM_DTYPE == "float8e4":
        return np.dtype(ml_dtypes.float8_e4m3)
    raise ValueError(MM_DTYPE)


def _build_nc(mm_dtype_name=None):
    mm_dtype_name = mm_dtype_name or MM_DTYPE
    import concourse.bacc as bacc
    import concourse.mybir as mybir
    import concourse.tile as tile

    mm_dt = getattr(mybir.dt, mm_dtype_name)
    fp8 = mm_dtype_name == "float8e4"
    # fp8 runs DoubleRow: 2 contraction sub-rows per partition per matmul
    ks = 2 if fp8 else 1
    nch = NLOC // (P * ks)  # contraction chunks per core
    nc = bacc.Bacc("TRN2", target_bir_lowering=False, debug=False,
                   num_devices=NCORES)
    # chunk-major input: each (layer, chunk) slice is one contiguous
    # region -> large DMA descriptors. Layout [P, ks, D] per chunk.
    x_in = nc.dram_tensor("x", [M, nch, P, ks, D], mm_dt,
                          kind="ExternalInput")
    cout = nc.dram_tensor("cout", [len(PAIRS), P, KBLK * D],
                          mybir.dt.float16, kind="ExternalOutput")

    perf_mode = mybir.MatmulPerfMode.DoubleRow if fp8 else None

    with tile.TileContext(nc) as tc:
        with (
            tc.tile_pool(name="xin", bufs=1) as xpool,
            tc.tile_pool(name="stage", bufs=3) as spool,
            tc.tile_pool(name="psum", bufs=8, space="PSUM") as ppool,
        ):
            # warmup: the PE sits idle ~3.5us waiting for the first input
            # DMA, and the HAM clock-gate needs >3us of sustained activity
            # to open to full clock. Run dummy matmuls on a zeroed scratch
            # tile during that window so the real stream starts warm.
            wsrc = xpool.tile([P, D], mm_dt, tag="warm")
            nc.gpsimd.memset(wsrc[:], 0.0)
            wacc = ppool.tile([P, D], mybir.dt.float32, tag="acc")
            for _ in range(6):
                nc.tensor.matmul(wacc[:], wsrc[:, :P], wsrc[:],
                                 start=True, stop=True)

            # one tile per (layer, chunk) so matmuls wait only on the
            # chunks they actually read — faster pipeline ramp
            xs = []
            ndma = 0
            # layers 0-1 gate the first matmuls: burst their 8 triggers
            # across all three DMA-capable engines so those transfers get
            # the fan-out bandwidth first. Layers 2-5 all go on gpsimd,
            # whose serial ~0.65us/trigger issue rate naturally paces
            # them behind the early layers without any semaphore gating.
            trig = [nc.sync, nc.gpsimd, nc.scalar]
            for i in range(M):
                row = []
                for c in range(nch):
                    xt = xpool.tile([P, ks, D], mm_dt, tag=f"x{i}c{c}")
                    eng = trig[ndma % 3] if i < 2 else nc.gpsimd
                    eng.dma_start(xt[:], x_in[i, c])
                    ndma += 1
                    row.append(xt)
                xs.append(row)

            ncopy = 0
            for pi, (i, j) in enumerate(PAIRS):
                st = spool.tile([P, KBLK * D], mybir.dt.float16, tag="st")
                off = 0
                for k in range(KBLK):
                    # C_ij rows [128k, 128k+128); for i==j skip column
                    # blocks left of the diagonal block.
                    e0 = k * P if i == j else 0
                    w = D - e0
                    acc = ppool.tile([P, D], mybir.dt.float32, tag="acc")
                    for c in range(nch):
                        nc.tensor.matmul(
                            acc[:, :w],
                            xs[i][c][:, :, k * P:(k + 1) * P],
                            xs[j][c][:, :, e0:],
                            start=(c == 0),
                            stop=(c == nch - 1),
                            perf_mode=perf_mode,
                        )
                    # spread PSUM->SBUF cast-copies over DVE and ACT
                    # (keep the final pair on DVE: lower latency tail)
                    dst = st[:, off:off + w]
                    last_pair = pi == len(PAIRS) - 1
                    if ncopy % 3 == 2 and not last_pair:
                        nc.scalar.copy(dst, acc[:, :w])
                    else:
                        nc.vector.tensor_copy(dst, acc[:, :w])
                    ncopy += 1
                    off += w
                nc.sync.dma_start(cout[pi, :, :off], st[:, :off])

    nc.compile()
    return nc


def _get_nc():
    if MM_DTYPE not in _CACHE:
        _CACHE[MM_DTYPE] = _build_nc()
    return _CACHE[MM_DTYPE]


def _shard_inputs(X):
    """X: [M, N, D] float32 -> per-core low-precision [M, nch, P, ks, D].

    Contraction sample (within a chunk) maps to (ks*128 + p) for fp8
    DoubleRow (ks=2), or just p otherwise (ks=1)."""
    Xh = X.astype(_np_mm_dtype())
    ks = 2 if MM_DTYPE == "float8e4" else 1
    nch = NLOC // (P * ks)
    maps = []
    for c in range(NCORES):
        sl = Xh[:, c * NLOC:(c + 1) * NLOC, :]
        sl = sl.reshape(M, nch, ks, P, D).transpose(0, 1, 3, 2, 4)
        maps.append({"x": np.ascontiguousarray(sl)})
    return maps


def _ensure_axon_hooks():
    """Some images lack antenv.axon_hooks; bass_utils imports it whenever
    tracing is requested (including via the BASS_TRACE env var). Provide a
    null hook module so that path degrades gracefully instead of raising."""
    try:
        from antenv.axon_hooks import get_axon_ntff_profile_hook  # noqa: F401
        return
    except ImportError:
        pass
    import sys
    import types

    try:
        import antenv
    except ImportError:
        return
    mod = types.ModuleType("antenv.axon_hooks")
    mod._hook = None
    mod.set_axon_ntff_profile_hook = lambda h: setattr(mod, "_hook", h)
    mod.get_axon_ntff_profile_hook = lambda: mod._hook
    sys.modules["antenv.axon_hooks"] = mod
    antenv.axon_hooks = mod


def _run_device(X, trace=False):
    _ensure_axon_hooks()
    from concourse.bass_utils import run_bass_kernel_spmd

    nc = _get_nc()
    res = run_bass_kernel_spmd(nc, _shard_inputs(X),
                               core_ids=list(range(NCORES)), trace=trace)
    return res


def _assemble(X, couts):
    """Host-side float64 assembly of the final CKA outputs.

    couts: per-core [21, P, KBLK*D] arrays of partial C blocks; block k
    of pair (i,j) has width w_k (= D, or D - 128k for diagonal pairs)
    and the blocks are packed back-to-back from column 0.
    """
    Csum = np.zeros((len(PAIRS), P, KBLK * D), np.float64)
    for c in couts:
        Csum += c.astype(np.float64)

    F2 = np.zeros((M, M))
    for pi, (i, j) in enumerate(PAIRS):
        if i != j:
            v = np.sum(Csum[pi] ** 2)
        else:
            v = 0.0
            off = 0
            for k in range(KBLK):
                w = D - k * P
                row = Csum[pi, :, off:off + w]
                v += np.sum(row[:, :P] ** 2)            # diagonal block (k,k)
                if w > P:
                    v += 2.0 * np.sum(row[:, P:] ** 2)  # blocks l > k
                off += w
        F2[i, j] = v
        F2[j, i] = v

    # use the same rounded X the device matmuls consumed, so the
    # g/r/t terms cancel the diagonal contributions of ||C||^2 exactly
    Xd = X.astype(_np_mm_dtype()).astype(np.float64)
    g = np.einsum("ind,ind->in", Xd, Xd)            # [M, N]
    s = Xd.sum(axis=1)                              # [M, D]
    r = np.einsum("ind,id->in", Xd, s) - g          # [M, N]
    t = r.sum(axis=1)                               # [M]

    S = F2 - g @ g.T - (2.0 / N) * (r @ r.T) + np.outer(t, t) / N ** 2
    S = np.abs(S)
    diag = np.diagonal(S)
    cka = S / np.sqrt(diag[:, None] * diag[None, :] + EPS)
    hsic = cka * np.tril(np.ones((M, M)), k=-1)
    l = np.sum(np.abs(hsic))
    vis = hsic + hsic.T + np.eye(M)
    return vis.astype(np.float32), np.array(l, dtype=np.float32)


def kernel(output):
    X = np.ascontiguousarray(np.asarray(output), dtype=np.float32)
    assert X.shape == (M, N, D)
    res = _run_device(X)
    couts = [r["cout"] for r in res.results]
    return _assemble(X, couts)


# revision 40
# speedup vs baseline: 1.0257x; 1.0018x over previous
"""CKA loss kernel for Trainium2 (8 NeuronCores, SPMD).

Math: for X_i = output[i] (shape [N, D]), the reference builds N x N Gram
matrices K_i = X_i X_i^T (diag zeroed), double-centers them and takes
pairwise inner products. Algebraically this reduces to D x D cross-Grams:

    S[i,j] = <Khat_i, Khat_j>
           = ||X_i^T X_j||_F^2 - g_i.g_j - (2/N) r_i.r_j + t_i t_j / N^2

with g_i[n] = ||X_i[n]||^2, r_i = X_i (X_i^T 1) - g_i, t_i = sum(r_i).
Only the ||X_i^T X_j||_F^2 term is heavy (C_ij = X_i^T X_j, 512x512,
contraction over N=4096) — everything else is O(M N D) and done on host
in float64.

Sharding: the contraction axis N is split across the 8 cores (512 samples
each). Every core computes partial C_ij for all 21 pairs (i<=j) from its
slice — perfectly balanced, no collectives. The host sums the partials in
float64, squares, and assembles the tiny [6,6] output.

The matmuls run in float16 (inputs are ~N(0,1); fp16 rounding of the
inputs perturbs S by ~1e-5 relative — the final cancellation is only ~8x).
Partial C blocks return to the host as float16 as well (entries are
O(100), and the per-entry rounding is ~2^-11 relative, far below the
tolerance). For diagonal pairs C_ii only column-blocks e >= d-block are
computed (block-level symmetry), saving ~12% of the matmul work.
"""

import numpy as np

M, N, D = 6, 4096, 512
NCORES = 8
NLOC = N // NCORES   # samples per core
P = 128              # partitions
NCH = NLOC // P      # contraction chunks per core
KBLK = D // P        # 128-row blocks of C
PAIRS = [(i, j) for i in range(M) for j in range(i, M)]  # 21
EPS = 1e-6

MM_DTYPE = "float16"  # matmul operand/output-staging dtype

_CACHE = {}


def _np_mm_dtype():
    if MM_DTYPE == "float16":
        return np.float16
    import ml_dtypes

    if MM_DTYPE == "bfloat16":
        return np.dtype(ml_dtypes.bfloat16)
    if MM_DTYPE == "float8e4":
        return np.dtype(ml_dtypes.float8_e4m3)
    raise ValueError(MM_DTYPE)


def _build_nc(mm_dtype_name=None):
    mm_dtype_name = mm_dtype_name or MM_DTYPE
    import concourse.bacc as bacc
    import concourse.mybir as mybir
    import concourse.tile as tile

    mm_dt = getattr(mybir.dt, mm_dtype_name)
    fp8 = mm_dtype_name == "float8e4"
    # fp8 runs DoubleRow: 2 contraction sub-rows per partition per matmul
    ks = 2 if fp8 else 1
    nch = NLOC // (P * ks)  # contraction chunks per core
    nc = bacc.Bacc("TRN2", target_bir_lowering=False, debug=False,
                   num_devices=NCORES)
    # chunk-major input: each (layer, chunk) slice is one contiguous
    # region -> large DMA descriptors. Layout [P, ks, D] per chunk.
    x_in = nc.dram_tensor("x", [M, nch, P, ks, D], mm_dt,
                          kind="ExternalInput")
    cout = nc.dram_tensor("cout", [len(PAIRS), P, KBLK * D],
                          mybir.dt.float16, kind="ExternalOutput")

    perf_mode = mybir.MatmulPerfMode.DoubleRow if fp8 else None

    with tile.TileContext(nc) as tc:
        with (
            tc.tile_pool(name="xin", bufs=1) as xpool,
            tc.tile_pool(name="stage", bufs=3) as spool,
            tc.tile_pool(name="psum", bufs=8, space="PSUM") as ppool,
        ):
            # warmup: the PE sits idle ~3.5us waiting for the first input
            # DMA, and the HAM clock-gate needs >3us of sustained activity
            # to open to full clock. Run dummy matmuls on a zeroed scratch
            # tile during that window so the real stream starts warm.
            wsrc = xpool.tile([P, D], mm_dt, tag="warm")
            nc.gpsimd.memset(wsrc[:], 0.0)
            wacc = ppool.tile([P, D], mybir.dt.float32, tag="acc")
            for _ in range(8):
                nc.tensor.matmul(wacc[:], wsrc[:, :P], wsrc[:],
                                 start=True, stop=True)

            # one tile per (layer, chunk) so matmuls wait only on the
            # chunks they actually read — faster pipeline ramp
            xs = []
            ndma = 0
            # rotate triggers over the three DMA-capable engines: each
            # DMA_DIRECT2D costs ~0.65us of issuing-engine queue time,
            # and the early stream is input-arrival limited
            trig = [nc.sync, nc.gpsimd, nc.scalar]
            for i in range(M):
                row = []
                for c in range(nch):
                    xt = xpool.tile([P, ks, D], mm_dt, tag=f"x{i}c{c}")
                    trig[ndma % 3].dma_start(xt[:], x_in[i, c])
                    ndma += 1
                    row.append(xt)
                xs.append(row)

            ncopy = 0
            for pi, (i, j) in enumerate(PAIRS):
                st = spool.tile([P, KBLK * D], mybir.dt.float16, tag="st")
                off = 0
                for k in range(KBLK):
                    # C_ij rows [128k, 128k+128); for i==j skip column
                    # blocks left of the diagonal block.
                    e0 = k * P if i == j else 0
                    w = D - e0
                    acc = ppool.tile([P, D], mybir.dt.float32, tag="acc")
                    for c in range(nch):
                        nc.tensor.matmul(
                            acc[:, :w],
                            xs[i][c][:, :, k * P:(k + 1) * P],
                            xs[j][c][:, :, e0:],
                            start=(c == 0),
                            stop=(c == nch - 1),
                            perf_mode=perf_mode,
                        )
                    # spread PSUM->SBUF cast-copies over DVE and ACT
                    # (keep the final pair on DVE: lower latency tail)
                    dst = st[:, off:off + w]
                    last_pair = pi == len(PAIRS) - 1
                    if ncopy % 3 == 2 and not last_pair:
                        nc.scalar.copy(dst, acc[:, :w])
                    else:
                        nc.vector.tensor_copy(dst, acc[:, :w])
                    ncopy += 1
                    off += w
                nc.sync.dma_start(cout[pi, :, :off], st[:, :off])

    nc.compile()
    return nc


def _get_nc():
    if MM_DTYPE not in _CACHE:
        _CACHE[MM_DTYPE] = _build_nc()
    return _CACHE[MM_DTYPE]


def _shard_inputs(X):
    """X: [M, N, D] float32 -> per-core low-precision [M, nch, P, ks, D].

    Contraction sample (within a chunk) maps to (ks*128 + p) for fp8
    DoubleRow (ks=2), or just p otherwise (ks=1)."""
    Xh = X.astype(_np_mm_dtype())
    ks = 2 if MM_DTYPE == "float8e4" else 1
    nch = NLOC // (P * ks)
    maps = []
    for c in range(NCORES):
        sl = Xh[:, c * NLOC:(c + 1) * NLOC, :]
        sl = sl.reshape(M, nch, ks, P, D).transpose(0, 1, 3, 2, 4)
        maps.append({"x": np.ascontiguousarray(sl)})
    return maps


def _ensure_axon_hooks():
    """Some images lack antenv.axon_hooks; bass_utils imports it whenever
    tracing is requested (including via the BASS_TRACE env var). Provide a
    null hook module so that path degrades gracefully instead of raising."""
    try:
        from antenv.axon_hooks import get_axon_ntff_profile_hook  # noqa: F401
        return
    except ImportError:
        pass
    import sys
    import types

    try:
        import antenv
    except ImportError:
        return
    mod = types.ModuleType("antenv.axon_hooks")
    mod._hook = None
    mod.set_axon_ntff_profile_hook = lambda h: setattr(mod, "_hook", h)
    mod.get_axon_ntff_profile_hook = lambda: mod._hook
    sys.modules["antenv.axon_hooks"] = mod
    antenv.axon_hooks = mod


def _run_device(X, trace=False):
    _ensure_axon_hooks()
    from concourse.bass_utils import run_bass_kernel_spmd

    nc = _get_nc()
    res = run_bass_kernel_spmd(nc, _shard_inputs(X),
                               core_ids=list(range(NCORES)), trace=trace)
    return res


def _assemble(X, couts):
    """Host-side float64 assembly of the final CKA outputs.

    couts: per-core [21, P, KBLK*D] arrays of partial C blocks; block k
    of pair (i,j) has width w_k (= D, or D - 128k for diagonal pairs)
    and the blocks are packed back-to-back from column 0.
    """
    Csum = np.zeros((len(PAIRS), P, KBLK * D), np.float64)
    for c in couts:
        Csum += c.astype(np.float64)

    F2 = np.zeros((M, M))
    for pi, (i, j) in enumerate(PAIRS):
        if i != j:
            v = np.sum(Csum[pi] ** 2)
        else:
            v = 0.0
            off = 0
            for k in range(KBLK):
                w = D - k * P
                row = Csum[pi, :, off:off + w]
                v += np.sum(row[:, :P] ** 2)            # diagonal block (k,k)
                if w > P:
                    v += 2.0 * np.sum(row[:, P:] ** 2)  # blocks l > k
                off += w
        F2[i, j] = v
        F2[j, i] = v

    # use the same rounded X the device matmuls consumed, so the
    # g/r/t terms cancel the diagonal contributions of ||C||^2 exactly
    Xd = X.astype(_np_mm_dtype()).astype(np.float64)
    g = np.einsum("ind,ind->in", Xd, Xd)            # [M, N]
    s = Xd.sum(axis=1)                              # [M, D]
    r = np.einsum("ind,id->in", Xd, s) - g          # [M, N]
    t = r.sum(axis=1)                               # [M]

    S = F2 - g @ g.T - (2.0 / N) * (r @ r.T) + np.outer(t, t) / N ** 2
    S = np.abs(S)
    diag = np.diagonal(S)
    cka = S / np.sqrt(diag[:, None] * diag[None, :] + EPS)
    hsic = cka * np.tril(np.ones((M, M)), k=-1)
    l = np.sum(np.abs(hsic))
    vis = hsic + hsic.T + np.eye(M)
    return vis.astype(np.float32), np.array(l, dtype=np.float32)


def kernel(output):
    X = np.ascontiguousarray(np.asarray(output), dtype=np.float32)
    assert X.shape == (M, N, D)
    res = _run_device(X)
    couts = [r["cout"] for r in res.results]
    return _assemble(X, couts)
